# revision 1
# baseline (speedup 1.0000x reference)
"""GQA attention kernel for Trainium2, 8 NeuronCores.

Problem: resid [2, 2048, 1024], 16 Q heads / 8 KV groups, d_head 64, causal,
out = softmax(QK^T/8 + causal) V -> W_out + b_out.

Sharding: tensor-parallel over (batch x kv-group-pairs). Core c handles
batch b = c // 4 and kv groups {2*(c%4), 2*(c%4)+1} = 4 Q heads. Each core
computes its heads' attention and a partial output projection; the host sums
the 4 partials per batch element and adds b_out.

Per-core dataflow, all matmul operands bf16 (fp32 PSUM accumulation):
  - warmup matmuls ramp the PE p-state while input DMAs land
  - Q^T, K^T, V^T projections with 512-wide moving dims; V^T is then
    PE-transposed in 128x128 blocks into V [k, e] layout with a ones
    column per group appended (memset once) so AV produces sum-exp free
  - scores computed transposed: S^T[k, q] = K @ Q^T; one matmul per
    (kv-group, k-tile) covers BOTH heads of the group via a strided rhs,
    writing a 2-bank PSUM "mega" so a single ACT exp instruction covers
    both heads (no Ln/Exp table thrash - reciprocal runs on DVE)
  - causality via q-start offset trimming + multiplicative upper-tri mask
    on diagonal tiles; no max-subtraction (scores are O(1))
  - U^T[e, (head, q)] += V_aug^T @ exp, one matmul per (group, k-tile)
    accumulating into a 2-bank PSUM pair
  - normalize: DVE reciprocal_approx_fast of sum row, gpsimd
    partition-broadcast, DVE multiply into z^T (bf16)
  - out_partial[s, d] = z^T.T @ W_out accumulated over 2 e-chunks,
    staged to SBUF bf16 and DMA'd out (host upcasts + sums partials)
  - projections of span n+1 are emitted before normalize/out-proj of
    span n so the PE stays busy across span boundaries
"""

import sys

sys.path.insert(0, "/opt/trn_rl_repo")

import ml_dtypes
import numpy as np

import concourse.bass as bass
import concourse.mybir as mybir
import concourse.tile as tile
from concourse import bacc
from concourse.bass_utils import run_bass_kernel_spmd
from concourse.masks import make_identity, make_upper_triangular

S = 2048          # seq len
D = 1024          # d_model
E = 64            # d_head
P = 128
NCHUNK = D // P   # 8 d_model chunks
SPAN = 512
NSPAN = S // SPAN
NKT = S // P      # 16 k tiles
F32 = mybir.dt.float32
BF16 = mybir.dt.bfloat16
EXP = mybir.ActivationFunctionType.Exp

LAST_RESULTS = None  # stashed BassKernelResults for the test harness
_CACHED_NC = None


def _build_program():
    nc = bacc.Bacc("TRN2", target_bir_lowering=False, debug=False)

    rT_d = nc.dram_tensor("resid_t", [D, S], BF16, kind="ExternalInput")
    # wq (256 cols, heads interleaved) | wk (128) | wv (128)
    wall_d = nc.dram_tensor("wall", [D, 512], BF16, kind="ExternalInput")
    wo_d = nc.dram_tensor("wo", [256, D], BF16, kind="ExternalInput")
    out_d = nc.dram_tensor("out", [S, D], BF16, kind="ExternalOutput")

    with tile.TileContext(nc) as tc:
        with (
            tc.tile_pool(name="persist", bufs=1) as pp,
            tc.tile_pool(name="exp", bufs=6) as ep,
            tc.tile_pool(name="vt", bufs=2) as vtp,
            tc.tile_pool(name="zt", bufs=2) as zp,
            tc.tile_pool(name="misc", bufs=2) as mp,
            tc.tile_pool(name="ostage", bufs=3) as op,
            tc.tile_pool(name="ps_u", bufs=1, space="PSUM") as ps_u,
            tc.tile_pool(name="ps_m", bufs=2, space="PSUM") as ps_m,
        ):
            # ---- constants (gpsimd, before its DMA queue work) ----
            mask = pp.tile([P, P], BF16, tag="mask")
            make_upper_triangular(nc, mask[:], val=1.0, diag=True)
            ident = pp.tile([P, P], BF16, tag="ident")
            make_identity(nc, ident[:])

            # V_aug [k, (group, 1|0..0|e)]: ones column 0, zeros 1:64, V at
            # 64:128. The sum-exp row of U lands at partition 0 (the custom
            # reciprocal mis-reads nonzero partition offsets) and the U block
            # at partitions 64:128 (DVE can only address PSUM at partition
            # offsets 0/64); written once
            vaug = [pp.tile([P, 2, P], BF16, tag=f"va{k}", name=f"va{k}")
                    for k in range(NKT)]
            for k in range(NKT):
                nc.vector.memset(vaug[k][:, :, 0:1], 1.0)
                nc.vector.memset(vaug[k][:, :, 1:E], 0.0)

            # ---- input DMAs, split across gpsimd/ACT/SP queues so the
            # startup burst isn't serialized on one sequencer ----
            wall_sb = []
            for c in range(NCHUNK):
                t = pp.tile([P, 512], BF16, tag=f"wall{c}")
                nc.gpsimd.dma_start(t[:], wall_d[c * P:(c + 1) * P, :])
                wall_sb.append(t)
            wo_sb = []
            for c in range(2):
                t = pp.tile([P, D], BF16, tag=f"wo{c}")
                nc.gpsimd.dma_start(t[:], wo_d[c * P:(c + 1) * P, :])
                wo_sb.append(t)

            # residual chunks, DMA'd span-wise so span 0 compute starts
            # after ~1MB instead of the full 4.2MB
            rT = []
            for c in range(NCHUNK):
                t = pp.tile([P, S], BF16, tag=f"rt{c}", name=f"rt{c}")
                rT.append(t)
            for sp in range(NSPAN):
                for c in range(NCHUNK):
                    nc.sync.dma_start(
                        rT[c][:, sp * SPAN:(sp + 1) * SPAN],
                        rT_d[c * P:(c + 1) * P, sp * SPAN:(sp + 1) * SPAN])

            qT = [pp.tile([P, S], BF16, tag=f"qt{e}", name=f"qt{e}")
                  for e in range(2)]
            kT = pp.tile([P, S], BF16, tag="kt")

            # ---- PE warmup: ramp the p-state while DMAs land ----
            wu = ps_m.tile([P, 2, SPAN], F32, tag="m", name="wu")
            for _ in range(40):
                nc.tensor.matmul(wu[:, 0, 0:P], ident[:], ident[:],
                                 start=True, stop=True,
                                 skip_group_check=True)

            def emit_proj(sp):
                q0 = sp * SPAN
                # Q projection: both head slots share a mega
                qmega = ps_m.tile([P, 2, SPAN], F32, tag="m", name="qmega")
                for e in range(2):
                    for c in range(NCHUNK):
                        nc.tensor.matmul(
                            qmega[:, e, :],
                            wall_sb[c][:, e * P:(e + 1) * P],
                            rT[c][:, q0:q0 + SPAN],
                            start=(c == 0),
                            stop=(c == NCHUNK - 1),
                        )
                for e in range(2):
                    nc.vector.tensor_copy(
                        qT[e][:, q0:q0 + SPAN], qmega[:, e, :])

                # K and V^T projections share a mega (bank 0 / bank 1)
                kvmega = ps_m.tile([P, 2, SPAN], F32, tag="m", name="kvmega")
                for half, col0 in ((0, 256), (1, 384)):
                    for c in range(NCHUNK):
                        nc.tensor.matmul(
                            kvmega[:, half, :],
                            wall_sb[c][:, col0:col0 + P],
                            rT[c][:, q0:q0 + SPAN],
                            start=(c == 0),
                            stop=(c == NCHUNK - 1),
                        )
                nc.vector.tensor_copy(kT[:, q0:q0 + SPAN], kvmega[:, 0, :])
                vTt = vtp.tile([P, SPAN], BF16, tag="vt", name="vTt")
                nc.vector.tensor_copy(vTt[:], kvmega[:, 1, :])

                # V^T -> V [k, e] via PE transposes, into vaug tiles
                vtr = ps_m.tile([P, 4, 2, E], BF16, tag="m", name="vtr")
                for j in range(4):
                    nc.tensor.transpose(
                        vtr[:, j], vTt[:, j * P:(j + 1) * P], ident[:])
                    nc.vector.tensor_copy(
                        vaug[4 * sp + j][:, :, E:2 * E], vtr[:, j])

            emit_proj(0)

            for sp in range(NSPAN):
                q0 = sp * SPAN
                nkt = (q0 + SPAN) // P  # k tiles touching this span

                # U accumulators: one bank per head slot (2g + i)
                u = [ps_u.tile([P, SPAN], F32, tag=f"u{j}", name=f"u{j}")
                     for j in range(4)]

                # software pipeline: AV of k-tile kt is emitted after the
                # scores+exp of kt+1, hiding the ACT exp latency from PE
                def emit_av(batch):
                    for g, e_t, kt_, off_, w_ in batch:
                        for i in range(2):
                            nc.tensor.matmul(
                                u[2 * g + i][0:P, off_:off_ + w_],
                                vaug[kt_][:, g, :],
                                e_t[:, i, off_:off_ + w_],
                                start=(kt_ == 0),
                                stop=(kt_ == nkt - 1),
                                skip_group_check=True,
                            )

                pending = []
                for kt in range(nkt):
                    k0 = kt * P
                    off = max(k0 - q0, 0)
                    w = SPAN - off
                    cur = []
                    for g in range(2):
                        meg = ps_m.tile([P, 2, SPAN], F32, tag="m", name="sc")
                        for i in range(2):
                            nc.tensor.matmul(
                                meg[:, i, off:off + w],
                                kT[g * E:(g + 1) * E, k0:k0 + P],
                                qT[i][g * E:(g + 1) * E,
                                      q0 + off:q0 + off + w],
                                start=True,
                                stop=True,
                                skip_group_check=True,
                            )
                        e_t = ep.tile([P, 2, SPAN], BF16, tag="e", name="e")
                        nc.scalar.activation(
                            e_t[:, :, off:off + w], meg[:, :, off:off + w],
                            EXP, scale=0.125,
                        )
                        if k0 >= q0:  # diagonal tile -> causal mask
                            for i in range(2):
                                nc.vector.tensor_mul(
                                    e_t[:, i, off:off + P],
                                    e_t[:, i, off:off + P],
                                    mask[:],
                                )
                        cur.append((g, e_t, kt, off, w))
                    emit_av(pending)
                    pending = cur
                emit_av(pending)

                # projections for the next span keep the PE busy while the
                # ACT exp tail and DVE normalize of this span drain
                if sp + 1 < NSPAN:
                    emit_proj(sp + 1)

                # normalize -> z^T chunks; zc[i] rows g*64 = head slot (g, i),
                # matching the host-side wo packing [h0, h2 | h1, h3]
                zc = [zp.tile([P, SPAN], BF16, tag=f"zt{i}", name=f"z{i}")
                      for i in range(2)]
                for g in range(2):
                    for i in range(2):
                        rec = mp.tile([1, SPAN], F32, tag="rec", name="rec")
                        nc.vector.reciprocal_approx_fast(
                            rec[:], u[2 * g + i][0:1, :])
                        bc = mp.tile([E, SPAN], F32, tag="bc", name="bc")
                        nc.gpsimd.partition_broadcast(bc[:], rec[:])
                        nc.vector.tensor_mul(
                            zc[i][g * E:(g + 1) * E, :],
                            u[2 * g + i][E:2 * E, :],
                            bc[:],
                        )

                # output projection for this span of s (o_ps reuses u banks)
                for st in range(4):
                    s0 = q0 + st * P
                    o_sb = op.tile([P, 2, SPAN], BF16, tag="ost")
                    for dsp in range(2):
                        o_ps = ps_u.tile([P, SPAN], F32,
                                         tag=f"u{(2 * st + dsp) % 4}",
                                         name="o_ps")
                        for ch in range(2):
                            nc.tensor.matmul(
                                o_ps[:],
                                zc[ch][:, st * P:(st + 1) * P],
                                wo_sb[ch][:, dsp * SPAN:(dsp + 1) * SPAN],
                                start=(ch == 0),
                                stop=(ch == 1),
                            )
                        nc.vector.tensor_copy(o_sb[:, dsp, :], o_ps[:])
                    # quarter-DMAs on two queues shorten the end-of-kernel
                    # drain after the last cast
                    for dsp in range(2):
                        for h in range(2):
                            eng = nc.sync if h == 0 else nc.gpsimd
                            c0 = dsp * SPAN + h * 256
                            eng.dma_start(
                                out_d[s0:s0 + P, c0:c0 + 256],
                                o_sb[:, dsp, h * 256:h * 256 + 256])

    nc.finalize()
    return nc


def kernel(resid, W_Q, W_K, W_V, W_out, b_out):
    global LAST_RESULTS, _CACHED_NC
    resid = np.asarray(resid, np.float32)
    W_Q = np.asarray(W_Q, np.float32)
    W_K = np.asarray(W_K, np.float32)
    W_V = np.asarray(W_V, np.float32)
    W_out = np.asarray(W_out, np.float32)
    b_out = np.asarray(b_out, np.float32)

    if _CACHED_NC is None:
        _CACHED_NC = _build_program()
    nc = _CACHED_NC

    bf16 = ml_dtypes.bfloat16
    residT = [resid[b].T.astype(bf16) for b in range(2)]
    in_maps = []
    for c in range(8):
        b, q = c // 4, c % 4
        # interleaved head order [h0, h2, h1, h3]: storage slot (g, i) holds
        # local head 2g+i -> qT[:, i]/zc[i] rows g*64 (see _build_program)
        heads = [4 * q, 4 * q + 2, 4 * q + 1, 4 * q + 3]
        groups = [2 * q, 2 * q + 1]
        wall = np.concatenate(
            [
                W_Q[:, heads, :].reshape(D, 256),
                W_K[:, groups, :].reshape(D, P),
                W_V[:, groups, :].reshape(D, P),
            ],
            axis=1,
        ).astype(bf16)
        in_maps.append({
            "resid_t": residT[b],
            "wall": wall,
            "wo": W_out[:, heads, :].transpose(1, 0, 2).reshape(256, D)
                  .astype(bf16),
        })

    res = run_bass_kernel_spmd(nc, in_maps, core_ids=list(range(8)))
    LAST_RESULTS = res

    out = np.zeros((2, S, D), np.float32)
    for c in range(8):
        out[c // 4] += res.results[c]["out"].astype(np.float32)
    out += b_out
    return out



# revision 8
# speedup vs baseline: 1.3122x; 1.3122x over previous
"""GQA attention kernel for Trainium2, 8 NeuronCores.

Problem: resid [2, 2048, 1024], 16 Q heads / 8 KV groups, d_head 64, causal,
out = softmax(QK^T/8 + causal) V -> W_out + b_out.

Sharding: tensor-parallel over (batch x kv-group-pairs). Core c handles
batch b = c // 4 and kv groups {2*(c%4), 2*(c%4)+1} = 4 Q heads. Each core
computes its heads' attention and a partial output projection; the host sums
the 4 partials per batch element and adds b_out.

Schedule (v2): the ACT exp stream is the pacing resource (~1.34us per
[128,2h,512] tile-pair); it runs continuously across span boundaries. The
PE interleaves scores with "filler" units (next-span Q/K/V projections,
out-projections of earlier spans) drawn between score tiles, while AV
accumulation is deferred a few k-tiles (deep e_t ring) so the four U PSUM
banks can host proj/out-proj megas in the window between spans. Causal
masks run on GpSimd (SBUF-only), normalization reciprocal+broadcast on
DVE+GpSimd, output staging casts on DVE, big 256KB output DMAs alternate
sync/gpsimd queues.

PSUM budget (8 banks): 4 banks = pu pool (tags b0..b3), time-shared per
span by [U accumulators -> out-proj o_ps -> next-next-span proj megas];
4 banks = pm pool (2 slots x 2 banks) exclusively for score megas so the
exp stream never waits on anything but its own WAR chain.
"""

import sys

sys.path.insert(0, "/opt/trn_rl_repo")

import ml_dtypes
import numpy as np

import concourse.bass as bass
import concourse.mybir as mybir
import concourse.tile as tile
from concourse import bacc
from concourse.bass_utils import run_bass_kernel_spmd
from concourse.masks import make_identity, make_upper_triangular

S = 2048          # seq len
D = 1024          # d_model
E = 64            # d_head
P = 128
NCHUNK = D // P   # 8 d_model chunks
SPAN = 512
NSPAN = S // SPAN
F32 = mybir.dt.float32
BF16 = mybir.dt.bfloat16
EXP = mybir.ActivationFunctionType.Exp

LAST_RESULTS = None  # stashed BassKernelResults for the test harness
_CACHED_NC = None


def _build_program():
    nc = bacc.Bacc("TRN2", target_bir_lowering=False, debug=False)

    rT_d = nc.dram_tensor("resid_t", [D, S], BF16, kind="ExternalInput")
    # wq (256 cols, heads interleaved) | wk (128) | wv (128)
    wall_d = nc.dram_tensor("wall", [D, 512], BF16, kind="ExternalInput")
    wo_d = nc.dram_tensor("wo", [256, D], BF16, kind="ExternalInput")
    out_d = nc.dram_tensor("out", [S, D], BF16, kind="ExternalOutput")

    with tile.TileContext(nc) as tc:
        with (
            tc.tile_pool(name="persist", bufs=1) as pp,
            tc.tile_pool(name="exp", bufs=20) as ep,
            tc.tile_pool(name="vt", bufs=2) as vtp,
            tc.tile_pool(name="zt", bufs=2) as zp,
            tc.tile_pool(name="misc", bufs=4) as mp,
            tc.tile_pool(name="ostage", bufs=4) as op,
            tc.tile_pool(name="ps_u", bufs=1, space="PSUM") as pu,
            tc.tile_pool(name="ps_m", bufs=2, space="PSUM") as pm,
        ):
            # ---- constants (gpsimd, before its DMA queue work) ----
            mask = pp.tile([P, P], BF16, tag="mask")
            make_upper_triangular(nc, mask[:], val=1.0, diag=True)
            ident = pp.tile([P, P], BF16, tag="ident")
            make_identity(nc, ident[:])

            # V_aug [k, (group, 1|0..0|e)]: ones column 0, zeros 1:64, V at
            # 64:128. The sum-exp row of U lands at partition 0 and the U
            # block at partitions 64:128 (DVE PSUM partition offsets 0/64
            # only); written once
            vaug = [pp.tile([P, 2, P], BF16, tag=f"va{k}", name=f"va{k}")
                    for k in range(S // P)]
            for k in range(S // P):
                nc.vector.memset(vaug[k][:, :, 0:1], 1.0)
                nc.vector.memset(vaug[k][:, :, 1:E], 0.0)

            # ---- input DMAs ----
            wall_sb = []
            for c in range(NCHUNK):
                t = pp.tile([P, 512], BF16, tag=f"wall{c}")
                nc.gpsimd.dma_start(t[:], wall_d[c * P:(c + 1) * P, :])
                wall_sb.append(t)
            wo_sb = []
            for c in range(2):
                t = pp.tile([P, D], BF16, tag=f"wo{c}")
                nc.gpsimd.dma_start(t[:], wo_d[c * P:(c + 1) * P, :])
                wo_sb.append(t)

            # residual chunks, DMA'd span-wise so span 0 compute starts
            # after ~1MB instead of the full 4.2MB
            rT = []
            for c in range(NCHUNK):
                t = pp.tile([P, S], BF16, tag=f"rt{c}", name=f"rt{c}")
                rT.append(t)
            for sp in range(NSPAN):
                for c in range(NCHUNK):
                    nc.sync.dma_start(
                        rT[c][:, sp * SPAN:(sp + 1) * SPAN],
                        rT_d[c * P:(c + 1) * P, sp * SPAN:(sp + 1) * SPAN])

            qT = [pp.tile([P, S], BF16, tag=f"qt{e}", name=f"qt{e}")
                  for e in range(2)]
            kT = pp.tile([P, S], BF16, tag="kt")

            # ---- PE warmup: ramp the p-state while DMAs land ----
            wu = pm.tile([P, 2, SPAN], F32, tag="m", name="wu")
            for _ in range(40):
                nc.tensor.matmul(wu[:, 0, 0:P], ident[:], ident[:],
                                 start=True, stop=True,
                                 skip_group_check=True)

            # causal-mask multiply (DVE; Pool rejects TensorScalarPtr in
            # the CoreV3 ISA check)
            def gmask_mul(dst, a, b):
                nc.vector.tensor_mul(dst, a, b)

            # ---------------- filler units ----------------
            # Each unit is a closure that emits a short PE burst (plus its
            # DVE/DMA companions). Units go through the pu bank ring.

            def unit_proj_q(sp, e, bank):
                def emit():
                    qmega = pu.tile([P, SPAN], F32, tag=f"b{bank}",
                                    name=f"qm{sp}_{e}")
                    q0 = sp * SPAN
                    for c in range(NCHUNK):
                        nc.tensor.matmul(
                            qmega[:],
                            wall_sb[c][:, e * P:(e + 1) * P],
                            rT[c][:, q0:q0 + SPAN],
                            start=(c == 0), stop=(c == NCHUNK - 1),
                        )
                    nc.vector.tensor_copy(qT[e][:, q0:q0 + SPAN], qmega[:])
                return emit

            def unit_proj_k(sp, bank):
                def emit():
                    kmega = pu.tile([P, SPAN], F32, tag=f"b{bank}",
                                    name=f"km{sp}")
                    q0 = sp * SPAN
                    for c in range(NCHUNK):
                        nc.tensor.matmul(
                            kmega[:],
                            wall_sb[c][:, 256:384],
                            rT[c][:, q0:q0 + SPAN],
                            start=(c == 0), stop=(c == NCHUNK - 1),
                        )
                    nc.vector.tensor_copy(kT[:, q0:q0 + SPAN], kmega[:])
                return emit

            def unit_proj_v(sp, bank, state):
                def emit():
                    vmega = pu.tile([P, SPAN], F32, tag=f"b{bank}",
                                    name=f"vm{sp}")
                    q0 = sp * SPAN
                    for c in range(NCHUNK):
                        nc.tensor.matmul(
                            vmega[:],
                            wall_sb[c][:, 384:512],
                            rT[c][:, q0:q0 + SPAN],
                            start=(c == 0), stop=(c == NCHUNK - 1),
                        )
                    vTt = vtp.tile([P, SPAN], BF16, tag="vt", name="vTt")
                    nc.vector.tensor_copy(vTt[:], vmega[:])
                    state["vTt"] = vTt
                return emit

            def unit_vtr(sp, bank, state):
                def emit():
                    vTt = state["vTt"]
                    vtr = pu.tile([P, 4, 2, E], BF16, tag=f"b{bank}",
                                  name=f"vtr{sp}")
                    for j in range(4):
                        nc.tensor.transpose(
                            vtr[:, j], vTt[:, j * P:(j + 1) * P], ident[:])
                        nc.vector.tensor_copy(
                            vaug[4 * sp + j][:, :, E:2 * E], vtr[:, j])
                return emit

            def unit_outproj(sp, st, zc, bank0):
                # one s-subtile: both d-halves + staging casts + DMAs
                def emit():
                    q0 = sp * SPAN
                    s0 = q0 + st * P
                    o_sb = op.tile([P, 2, SPAN], BF16, tag="ost",
                                   name=f"osb{sp}_{st}")
                    eng = nc.sync if st % 2 == 0 else nc.gpsimd
                    for dsp in range(2):
                        o_ps = pu.tile([P, SPAN], F32,
                                       tag=f"b{(bank0 + dsp) % 4}",
                                       name="o_ps")
                        for ch in range(2):
                            nc.tensor.matmul(
                                o_ps[:],
                                zc[ch][:, st * P:(st + 1) * P],
                                wo_sb[ch][:, dsp * SPAN:(dsp + 1) * SPAN],
                                start=(ch == 0), stop=(ch == 1),
                            )
                        nc.vector.tensor_copy(o_sb[:, dsp, :], o_ps[:])
                        eng.dma_start(
                            out_d[s0:s0 + P, dsp * SPAN:(dsp + 1) * SPAN],
                            o_sb[:, dsp, :])
                return emit

            def emit_proj_units(sp):
                state = {}
                return [
                    unit_proj_q(sp, 0, 0),
                    unit_proj_q(sp, 1, 1),
                    unit_proj_k(sp, 2),
                    unit_proj_v(sp, 3, state),
                    unit_vtr(sp, 3, state),
                ]

            def emit_normalize(sp, u):
                # u banks -> zc (SBUF bf16); returns zc pair
                zc = [zp.tile([P, SPAN], BF16, tag=f"zt{i}", name=f"z{sp}_{i}")
                      for i in range(2)]
                for g in range(2):
                    for i in range(2):
                        rec = mp.tile([1, SPAN], F32, tag="rec", name="rec")
                        nc.vector.reciprocal_approx_fast(
                            rec[:], u[2 * g + i][0:1, :])
                        bc = mp.tile([E, SPAN], F32, tag="bc", name="bc")
                        nc.gpsimd.partition_broadcast(bc[:], rec[:])
                        nc.vector.tensor_mul(
                            zc[i][g * E:(g + 1) * E, :],
                            u[2 * g + i][E:2 * E, :],
                            bc[:],
                        )
                return zc

            # ---------------- span 0 projections (not fillers) ----------------
            for u_ in emit_proj_units(0):
                u_()

            # state carried across spans
            zc_of = {}        # sp -> zc pair
            u_of = {}         # sp -> u bank tiles

            for sp in range(NSPAN):
                q0 = sp * SPAN
                nkt = (q0 + SPAN) // P  # k tiles touching this span

                # normalize previous span (DVE/gpsimd; depends on av tail)
                if sp > 0:
                    zc_of[sp - 1] = emit_normalize(sp - 1, u_of[sp - 1])

                # filler units for this span's attention window:
                #   proj(sp+1)  (must finish this span)
                #   outproj(sp-2) and, in span 3, also outproj(sp-1)
                fillers = []
                if sp + 1 < NSPAN:
                    fillers += emit_proj_units(sp + 1)
                if sp == 2:
                    fillers += [unit_outproj(0, st, zc_of[0], 2 * st % 4)
                                for st in range(4)]
                if sp == 3:
                    fillers += [unit_outproj(1, st, zc_of[1], 2 * st % 4)
                                for st in range(4)]
                    fillers += [unit_outproj(2, st, zc_of[2], 2 * st % 4)
                                for st in range(4)]

                # spread fillers over the first ktiles, ~1 per ktile
                fill_iter = iter(fillers)

                u = None
                pending_av = []   # (kt, off, w, [(g, e_t)])
                done_fillers = False

                def emit_av(entry, u, nkt):
                    kt_, off_, w_, gs = entry
                    for g, e_t in gs:
                        for i in range(2):
                            nc.tensor.matmul(
                                u[2 * g + i][0:P, off_:off_ + w_],
                                vaug[kt_][:, g, :],
                                e_t[:, i, off_:off_ + w_],
                                start=(kt_ == 0),
                                stop=(kt_ == nkt - 1),
                                skip_group_check=True,
                            )

                for kt in range(nkt):
                    k0 = kt * P
                    off = max(k0 - q0, 0)
                    w = SPAN - off
                    gs = []
                    for g in range(2):
                        meg = pm.tile([P, 2, SPAN], F32, tag="m", name="sc")
                        for i in range(2):
                            nc.tensor.matmul(
                                meg[:, i, off:off + w],
                                kT[g * E:(g + 1) * E, k0:k0 + P],
                                qT[i][g * E:(g + 1) * E,
                                      q0 + off:q0 + off + w],
                                start=True, stop=True,
                                skip_group_check=True,
                            )
                        e_t = ep.tile([P, 2, SPAN], BF16, tag="e", name="e")
                        nc.scalar.activation(
                            e_t[:, :, off:off + w], meg[:, :, off:off + w],
                            EXP, scale=0.125,
                        )
                        if k0 >= q0:  # diagonal tile -> causal mask (gpsimd)
                            for i in range(2):
                                gmask_mul(
                                    e_t[:, i, off:off + P],
                                    e_t[:, i, off:off + P],
                                    mask[:],
                                )
                        gs.append((g, e_t))
                    pending_av.append((kt, off, w, gs))

                    # one filler unit per ktile until exhausted
                    nxt = next(fill_iter, None)
                    if nxt is not None:
                        nxt()
                    else:
                        if not done_fillers:
                            done_fillers = True
                            # claim u banks now and drain the av backlog
                            u = [pu.tile([P, SPAN], F32, tag=f"b{j}",
                                         name=f"u{sp}_{j}")
                                 for j in range(4)]
                            u_of[sp] = u
                        # catch up: emit oldest pending avs (keep lag >= 2)
                        while len(pending_av) > 2:
                            emit_av(pending_av.pop(0), u, nkt)

                # drain any fillers that didn't fit in the ktile loop, then
                # claim the u banks and finish the av backlog
                for f in fill_iter:
                    f()
                if u is None:
                    u = [pu.tile([P, SPAN], F32, tag=f"b{j}",
                                 name=f"u{sp}_{j}") for j in range(4)]
                    u_of[sp] = u
                while pending_av:
                    emit_av(pending_av.pop(0), u, nkt)

            # ---------------- tail: last span normalize + outproj ----------------
            zc_of[3] = emit_normalize(3, u_of[3])
            for st in range(4):
                unit_outproj(3, st, zc_of[3], 2 * st % 4)()

    nc.finalize()
    return nc


def kernel(resid, W_Q, W_K, W_V, W_out, b_out):
    global LAST_RESULTS, _CACHED_NC
    resid = np.asarray(resid, np.float32)
    W_Q = np.asarray(W_Q, np.float32)
    W_K = np.asarray(W_K, np.float32)
    W_V = np.asarray(W_V, np.float32)
    W_out = np.asarray(W_out, np.float32)
    b_out = np.asarray(b_out, np.float32)

    if _CACHED_NC is None:
        _CACHED_NC = _build_program()
    nc = _CACHED_NC

    bf16 = ml_dtypes.bfloat16
    residT = [resid[b].T.astype(bf16) for b in range(2)]
    in_maps = []
    for c in range(8):
        b, q = c // 4, c % 4
        # interleaved head order [h0, h2, h1, h3]: storage slot (g, i) holds
        # local head 2g+i -> qT[:, i]/zc[i] rows g*64 (see _build_program)
        heads = [4 * q, 4 * q + 2, 4 * q + 1, 4 * q + 3]
        groups = [2 * q, 2 * q + 1]
        wall = np.concatenate(
            [
                W_Q[:, heads, :].reshape(D, 256),
                W_K[:, groups, :].reshape(D, P),
                W_V[:, groups, :].reshape(D, P),
            ],
            axis=1,
        ).astype(bf16)
        in_maps.append({
            "resid_t": residT[b],
            "wall": wall,
            "wo": W_out[:, heads, :].transpose(1, 0, 2).reshape(256, D)
                  .astype(bf16),
        })

    res = run_bass_kernel_spmd(nc, in_maps, core_ids=list(range(8)))
    LAST_RESULTS = res

    out = np.zeros((2, S, D), np.float32)
    for c in range(8):
        out[c // 4] += res.results[c]["out"].astype(np.float32)
    out += b_out
    return out
